# revision 1
# baseline (speedup 1.0000x reference)
"""Self-contained Trainium2 Bass kernel for nn_EnhancedGNN (4-layer GNN:
SAGEConv -> GINConv -> SAGEConv -> SAGEConv with BatchNorm + log_softmax)
on 8 NeuronCores.

kernel(**inputs) takes the FULL unsharded inputs (x [100000,128] f32,
edge_index [2,1600000] int32, weights...) and returns the FULL
[100000, 40] f32 log-softmax output.

Distribution: nodes sharded 8 ways (edges partitioned by destination
owner), small weights replicated, and a full fp16 copy of the node
features kept per core, refreshed by AllGather after each layer.
Per-edge aggregation = dma_gather of source rows (fp16, 256B each,
4 SWDGE queues) + one-hot matmul scatter-add accumulated in PSUM.
On-device compute is feature-major (features on partitions) so BatchNorm
is a per-partition affine and all dense matmuls are weight-stationary.
BN batch statistics are AllReduced across cores.
"""
import dataclasses
import sys
import types
import numpy as np

# ---------------------------------------------------------------------------
# harness patches (walrus on this image encodes at most ONE sync wait per
# instruction; the axon NTFF profile hook is missing from the shipped antenv)
# ---------------------------------------------------------------------------


def _apply_tile_drain_patch():
    import concourse.tile as tile_mod
    from concourse.vector_clock import ScopedClock, VectorClock

    def _patched(self, tick_clock, wait_clock):
        nc = self.nc
        gc = tick_clock.global_clock
        n = len(gc)
        for i in range(n):
            t = gc[i]
            if t <= 0:
                continue
            vec = [0] * n
            vec[i] = t
            d = nc.sync.drain()
            wait_clock.add_sem_waits(d.ins, ScopedClock({None: VectorClock(vec)}))
        nc.sync.drain()
        nc.all_engine_barrier()
        assert self.sems is not None
        popped = nc._tile_sem_poison_stack.pop()
        assert popped is self._sem_poison
        nc.clear_and_free_semaphores(list(self.sems.allocated().values()))
        nc.all_engine_barrier()

    tile_mod.TileContext._drain_and_barrier = _patched


def _split_sync_waits(nc, max_waits=1):
    import concourse.mybir as mybir
    n_split = 0
    for f in nc.m.functions:
        for blk in f.blocks:
            new_insts = []
            for ins in blk.instructions:
                si = ins.sync_info
                if si is not None and si.on_wait and len(si.on_wait) > max_waits:
                    waits = list(si.on_wait)
                    keep = waits[-max_waits:]
                    for w in waits[:-max_waits]:
                        nop = mybir.InstNoOp(
                            name=f"{ins.name}-ws{n_split}", ins=[], outs=[])
                        nop.engine = ins.engine
                        nop.sync_info = mybir.SyncInfo(on_wait=[w], on_update=[])
                        new_insts.append(nop)
                        n_split += 1
                    si.on_wait = keep
                new_insts.append(ins)
            blk.instructions[:] = new_insts
    return n_split


def _install_ntff_hook():
    if 'antenv.axon_hooks' in sys.modules:
        return
    try:
        from trn_agent_boot.trn_boot import _ntff_profile_via_ctypes
        hook = _ntff_profile_via_ctypes('/opt/axon/libaxon_pjrt.so')
    except Exception:
        hook = None
    mod = types.ModuleType('antenv.axon_hooks')
    state = {'hook': hook}
    mod.get_axon_ntff_profile_hook = lambda: state['hook']
    mod.set_axon_ntff_profile_hook = lambda h: state.update(hook=h)
    sys.modules['antenv.axon_hooks'] = mod
    try:
        import antenv
        antenv.axon_hooks = mod
    except Exception:
        pass
    try:
        import concourse.bass_utils as bu
        bu.upload_artifacts = lambda tmpdir: tmpdir
    except Exception:
        pass


def _maybe_reset_device():
    try:
        import jax, ctypes
        jax.devices()
        lib = ctypes.CDLL('/opt/axon/libaxon_pjrt.so')
        lib.axon_reset.restype = ctypes.c_int64
        lib.axon_reset()
    except Exception:
        pass


# ---------------------------------------------------------------------------
# kernel build
# ---------------------------------------------------------------------------

H = 128
DOUT = 40
BN_EPS = 1e-5
NCORES = 8
P = 128


@dataclasses.dataclass
class Plan:
    N: int; E: int; OWN: int; OWNP: int; NP: int; T: int; B: int; NB: int
    C_total: int; ncalls: int
    call_bucket: list; tile_ranges: list
    idx_w: list; ds: list; invdeg: list; xT: list; xpad: np.ndarray


def build_plan(x, edge_index, bucket=32768):
    N, D = x.shape
    E = edge_index.shape[1]
    OWN = N // NCORES
    OWNP = -(-OWN // P) * P
    NP = OWNP * NCORES
    T = OWNP // P
    B = bucket
    NB = -(-NP // B)
    src, dst = edge_index[0].astype(np.int64), edge_index[1].astype(np.int64)
    srcp = src + (OWNP - OWN) * (src // OWN)
    owner = dst // OWN
    dl = dst - owner * OWN
    bkt = srcp // B

    per_core = []
    counts = np.zeros((NCORES, NB, T), np.int64)
    for c in range(NCORES):
        m = owner == c
        sp, d, b = srcp[m], dl[m], bkt[m]
        t = d // P
        order = np.lexsort((d, t, b))
        sp, d, b, t = sp[order], d[order], b[order], t[order]
        np.add.at(counts[c], (b, t), 1)
        per_core.append((sp, d, b, t))

    seg = -(-counts.max(axis=0) // P) * P
    cb = seg.sum(axis=1) // P
    cb_pad = -(-cb // 8) * 8
    C_total = int(cb_pad.sum())
    ncalls = C_total // 8

    chunk_tile = []
    tile_ranges = [[None] * T for _ in range(NB)]
    for b in range(NB):
        for t in range(T):
            nch = int(seg[b, t]) // P
            tile_ranges[b][t] = (len(chunk_tile), nch)
            chunk_tile += [t] * nch
        chunk_tile += [-1] * int(cb_pad[b] - cb[b])
    call_bucket = []
    s = np.cumsum(cb_pad)
    for g in range(ncalls):
        call_bucket.append(int(np.searchsorted(s, g * 8, side='right')))

    idx_ws, dss, invs, xTs = [], [], [], []
    deg = np.bincount(dst, minlength=N).astype(np.float32)
    inv_all = 1.0 / np.maximum(deg, 1.0)
    for c in range(NCORES):
        sp, d, b, t = per_core[c]
        idxv = np.zeros(C_total * P, np.int32)
        dsv = np.full(C_total * P, -1.0, np.float32)
        for bb in range(NB):
            for tt in range(T):
                m = (b == bb) & (t == tt)
                k = int(m.sum())
                c0, _ = tile_ranges[bb][tt]
                base = c0 * P
                idxv[base:base + k] = sp[m] - bb * B
                dsv[base:base + k] = d[m] - tt * P
        v = idxv.reshape(ncalls, 64, 16)
        w16 = np.transpose(v, (2, 0, 1)).reshape(16, ncalls * 64)
        idx_ws.append(np.tile(w16, (8, 1)).astype(np.int16))
        dss.append(np.ascontiguousarray(
            dsv.reshape(C_total, P).T).astype(np.float16))
        iv = np.zeros(OWNP, np.float32)
        iv[:OWN] = inv_all[c * OWN:(c + 1) * OWN]
        invs.append(np.tile(iv[None, :], (P, 1)).astype(np.float16))
        xo = np.zeros((OWNP, D), np.float32)
        xo[:OWN] = x[c * OWN:(c + 1) * OWN]
        xTs.append(np.ascontiguousarray(xo.T).astype(np.float16))

    xpad = np.zeros((NP, D), np.float16)
    for c in range(NCORES):
        xpad[c * OWNP:c * OWNP + OWN] = x[c * OWN:(c + 1) * OWN].astype(
            np.float16)

    return Plan(N=N, E=E, OWN=OWN, OWNP=OWNP, NP=NP, T=T, B=B, NB=NB,
                C_total=C_total, ncalls=ncalls, call_bucket=call_bucket,
                tile_ranges=tile_ranges, idx_w=idx_ws, ds=dss, invdeg=invs,
                xT=xTs, xpad=xpad)


def _bcast_mid(ap, reps):
    return dataclasses.replace(ap, ap=[ap.ap[0], [0, reps], ap.ap[1]])


def _bcast_last(ap, reps):
    return dataclasses.replace(ap, ap=[ap.ap[0], ap.ap[1], [0, reps]])


def build_bass(pl):
    import concourse.bass as bass
    import concourse.mybir as mybir
    from concourse.tile import TileContext
    from concourse import library_config
    from concourse.library_overlay import lower_extended_insts

    F16, F32, I16 = mybir.dt.float16, mybir.dt.float32, mybir.dt.int16
    AX = mybir.AxisListType
    ALU = mybir.AluOpType
    ACTF = mybir.ActivationFunctionType

    nc = bass.Bass('TRN2', target_bir_lowering=False, debug=False,
                   num_devices=NCORES, num_swdge_queues=4)

    def din(name, shape, dt):
        return nc.dram_tensor(name, shape, dt, kind='ExternalInput')

    xpad_d = din('xpad', [pl.NP, H], F16)
    xT_d = din('xT', [P, pl.OWNP], F16)
    idx_d = din('idxw', [P, pl.ncalls * 64], I16)
    ds_d = din('ds', [P, pl.C_total], F16)
    inv_d = din('invdeg', [P, pl.OWNP], F16)
    iota_d = din('iota', [P, P], F16)
    ident_d = din('ident', [P, P], F16)
    wname = ['w0l', 'w0r', 'w1', 'w2', 'w2l', 'w2r']
    w_d = {k: din(k, [H, H], F16) for k in wname}
    fwl_d = din('fwl', [H, DOUT], F16)
    fwr_d = din('fwr', [H, DOUT], F16)
    b1_d = din('b1T', [P, 1], F32)
    finb_d = din('finb', [P, DOUT], F32)
    bng_d = din('bngT', [P, 3], F32)
    bnb_d = din('bnbT', [P, 3], F32)
    OUTP = 64
    out_d = nc.dram_tensor('out', [pl.OWNP, OUTP], F32, kind='ExternalOutput')

    hown_d = nc.dram_tensor('hown', [pl.OWNP, H], F16, kind='Internal')
    hfull_d = nc.dram_tensor('hfull', [pl.NP, H], F16, kind='Internal')
    stin_d = [nc.dram_tensor(f'stin{i}', [P, 2], F32, kind='Internal')
              for i in range(3)]
    stout_d = [nc.dram_tensor(f'stout{i}', [P, 2], F32, kind='Internal',
                              addr_space='Shared') for i in range(3)]
    rg = [list(range(NCORES))]

    nc.gpsimd.load_library(library_config.mlp)

    NGRP = -(-pl.OWNP // 512)
    grp_w = [min(512, pl.OWNP - g * 512) for g in range(NGRP)]
    grp_v = [min(512, max(0, pl.OWN - g * 512)) for g in range(NGRP)]

    with TileContext(nc) as tc:
        nidx_reg = nc.gpsimd.to_reg(1024)
        import contextlib
        ctx = contextlib.ExitStack()
        with ctx:
            persist = ctx.enter_context(tc.tile_pool(name='persist', bufs=1))
            gpool = ctx.enter_context(tc.tile_pool(name='g', bufs=12))
            spool = ctx.enter_context(tc.tile_pool(name='s', bufs=12))
            ipool = ctx.enter_context(tc.tile_pool(name='idx', bufs=12))
            epool = ctx.enter_context(tc.tile_pool(name='evac', bufs=3))
            psA = ctx.enter_context(
                tc.tile_pool(name='psA', bufs=3, space='PSUM'))
            psD = ctx.enter_context(
                tc.tile_pool(name='psD', bufs=2, space='PSUM'))
            psT = ctx.enter_context(
                tc.tile_pool(name='psT', bufs=2, space='PSUM'))

            def load(name, shape, dt, src):
                t = persist.tile(shape, dt, tag=name)
                nc.sync.dma_start(t[:], src)
                return t

            ds_sb = load('ds', [P, pl.C_total], F16, ds_d[:])
            inv_sb = load('inv', [P, pl.OWNP], F16, inv_d[:])
            iota_sb = load('iota', [P, P], F16, iota_d[:])
            ident_sb = load('ident', [P, P], F16, ident_d[:])
            w_sb = {k: load(k, [H, H], F16, w_d[k][:]) for k in wname}
            fwl_sb = load('fwl', [H, DOUT], F16, fwl_d[:])
            fwr_sb = load('fwr', [H, DOUT], F16, fwr_d[:])
            b1_sb = load('b1', [P, 1], F32, b1_d[:])
            finb_sb = load('finb', [P, DOUT], F32, finb_d[:])
            bng_sb = load('bng', [P, 3], F32, bng_d[:])
            bnb_sb = load('bnb', [P, 3], F32, bnb_d[:])
            xq = load('xq', [P, pl.OWNP], F16, xT_d[:])

            hA = persist.tile([P, pl.OWNP], F16, tag='hA')
            hB = persist.tile([P, pl.OWNP], F16, tag='hB')
            aggT = persist.tile([P, pl.OWNP], F16, tag='aggT')
            sums = persist.tile([P, NGRP], F32, tag='sums')
            sqs = persist.tile([P, NGRP], F32, tag='sqs')
            stat = persist.tile([P, 2], F32, tag='stat')
            gstat = persist.tile([P, 2], F32, tag='gstat')
            scl = persist.tile([P, 1], F32, tag='scl')
            bia = persist.tile([P, 1], F32, tag='bia')
            tmp1 = persist.tile([P, 1], F32, tag='tmp1')
            tmp2 = persist.tile([P, 1], F32, tag='tmp2')
            ttscr = persist.tile([P, 512], F32, tag='ttscr')
            logit = persist.tile([P, pl.T * DOUT], F32, tag='logit')
            mx = persist.tile([P, pl.T], F32, tag='mx')
            lse = persist.tile([P, pl.T], F32, tag='lse')
            escr = persist.tile([P, DOUT], F32, tag='escr')

            def aggregate(table_d, gin, cur_h, group_cb=None):
                grp_done = 0

                def finish_group(g):
                    if not gin:
                        w = grp_w[g]
                        sl = aggT[:, g * 512:g * 512 + w]
                        nc.vector.tensor_tensor(
                            out=sl, in0=sl,
                            in1=inv_sb[:, g * 512:g * 512 + w], op=ALU.mult)
                    if group_cb is not None:
                        group_cb(g)

                call = 0
                tile_init = set()
                border = list(range(pl.NB - 1, -1, -1))
                for bi, b in enumerate(border):
                    last_b = bi == pl.NB - 1
                    rows = min(pl.B, pl.NP - b * pl.B)
                    tab = table_d[b * pl.B: b * pl.B + rows, :]
                    bcalls = [g for g in range(pl.ncalls)
                              if pl.call_bucket[g] == b]
                    gtiles = {}
                    emitted = 0

                    def emit_call(k):
                        nonlocal call, emitted
                        g = bcalls[k]
                        it = ipool.tile([P, 64], I16, tag='idx')
                        nc.sync.dma_start(it[:], idx_d[:, g * 64:(g + 1) * 64])
                        gt = gpool.tile([P, 8, P], F16, tag='g')
                        nc.gpsimd.dma_gather(
                            gt[:], tab, it[:], num_idxs=1024,
                            num_idxs_reg=nidx_reg, elem_size=H,
                            queue_num=call % 4)
                        st = spool.tile([P, 8 * P], F16, tag='s')
                        nc.vector.tensor_tensor(
                            out=st[:].rearrange('p (c f) -> p c f', c=8),
                            in0=_bcast_mid(iota_sb[:], 8),
                            in1=_bcast_last(ds_sb[:, g * 8:(g + 1) * 8], P),
                            op=ALU.is_equal)
                        gtiles[g] = (gt, st)
                        call += 1
                        emitted += 1

                    base_call = bcalls[0]
                    AHEAD = 4
                    for k in range(min(AHEAD, len(bcalls))):
                        emit_call(k)
                    for t in range(pl.T):
                        c0, nch = pl.tile_ranges[b][t]
                        if nch == 0:
                            if last_b:
                                while grp_done < NGRP and \
                                        (t + 1) * P >= min(
                                            (grp_done + 1) * 512, pl.OWNP):
                                    finish_group(grp_done)
                                    grp_done += 1
                            continue
                        need = (c0 + nch - 1) // 8 - base_call + 1
                        while emitted < min(need + AHEAD, len(bcalls)):
                            emit_call(emitted)
                        pt = psA.tile([P, P], F32, tag='agg')
                        extra = 1 if (gin and t not in tile_init) else 0
                        for j in range(nch):
                            ch = c0 + j
                            gt, st = gtiles[ch // 8]
                            jj = ch % 8
                            nc.tensor.matmul(
                                pt[:], lhsT=gt[:, jj, :],
                                rhs=st[:, jj * P:(jj + 1) * P],
                                start=(j == 0),
                                stop=(j == nch - 1 and not extra),
                                skip_group_check=True)
                        if extra:
                            nc.tensor.matmul(
                                pt[:], lhsT=ident_sb[:],
                                rhs=cur_h[:, t * P:(t + 1) * P],
                                start=False, stop=True,
                                skip_group_check=True)
                        sl = aggT[:, t * P:(t + 1) * P]
                        if t not in tile_init:
                            tile_init.add(t)
                            nc.vector.tensor_copy(out=sl, in_=pt[:])
                        else:
                            nc.vector.tensor_tensor(
                                out=sl, in0=sl, in1=pt[:], op=ALU.add)
                    if last_b:
                        # fire per-group post-processing as tiles finalize
                        while grp_done < NGRP and \
                                (t + 1) * P >= min((grp_done + 1) * 512,
                                                   pl.OWNP):
                            finish_group(grp_done)
                            grp_done += 1
                    while emitted < len(bcalls):
                        emit_call(emitted)
                while grp_done < NGRP:
                    finish_group(grp_done)
                    grp_done += 1

            def dense_prebn(layer, g, cur):
                w = grp_w[g]
                pt = psD.tile([P, 512], F32, tag='dense')
                sl = slice(g * 512, g * 512 + w)
                if layer == 0:
                    nc.tensor.matmul(pt[:, :w], lhsT=w_sb['w0l'][:],
                                     rhs=aggT[:, sl], start=True, stop=False,
                                     skip_group_check=True)
                    nc.tensor.matmul(pt[:, :w], lhsT=w_sb['w0r'][:],
                                     rhs=xq[:, sl], start=False, stop=True,
                                     skip_group_check=True)
                elif layer == 1:
                    nc.tensor.matmul(pt[:, :w], lhsT=w_sb['w2'][:],
                                     rhs=xq[:, sl], start=True, stop=True,
                                     skip_group_check=True)
                else:
                    nc.tensor.matmul(pt[:, :w], lhsT=w_sb['w2l'][:],
                                     rhs=aggT[:, sl], start=True, stop=False,
                                     skip_group_check=True)
                    nc.tensor.matmul(pt[:, :w], lhsT=w_sb['w2r'][:],
                                     rhs=cur[:, sl], start=False, stop=True,
                                     skip_group_check=True)
                return pt

            def stats_pass1(layer, cur):
                def cb(g):
                    w, v = grp_w[g], grp_v[g]
                    pt = dense_prebn(layer, g, cur)
                    if v > 0:
                        nc.vector.tensor_reduce(
                            out=sums[:, g:g + 1], in_=pt[:, :v], axis=AX.X,
                            op=ALU.add)
                        nc.scalar.activation(
                            ttscr[:, :v], pt[:, :v], ACTF.Square,
                            accum_out=sqs[:, g:g + 1])
                    else:
                        nc.vector.memset(sums[:, g:g + 1], 0.0)
                        nc.vector.memset(sqs[:, g:g + 1], 0.0)
                return cb

            def bn_stats_and_apply(layer, cur, nxt, residual, writeback=True):
                nc.vector.tensor_reduce(out=stat[:, 0:1], in_=sums[:],
                                        axis=AX.X, op=ALU.add)
                nc.vector.tensor_reduce(out=stat[:, 1:2], in_=sqs[:],
                                        axis=AX.X, op=ALU.add)
                nc.sync.dma_start(stin_d[layer][:], stat[:])
                nc.gpsimd.collective_compute(
                    'AllReduce', ALU.add, rg, ins=[stin_d[layer][:]],
                    outs=[stout_d[layer][:]])
                nc.sync.dma_start(gstat[:], stout_d[layer][:])
                invN = 1.0 / float(pl.N)
                nc.vector.tensor_scalar_mul(tmp1[:], gstat[:, 0:1], invN)
                nc.vector.tensor_scalar_mul(tmp2[:], gstat[:, 1:2], invN)
                nc.vector.tensor_tensor(out=scl[:], in0=tmp1[:], in1=tmp1[:],
                                        op=ALU.mult)
                nc.vector.tensor_tensor(out=tmp2[:], in0=tmp2[:], in1=scl[:],
                                        op=ALU.subtract)
                nc.vector.tensor_scalar_add(tmp2[:], tmp2[:], BN_EPS)
                nc.scalar.sqrt(tmp2[:], tmp2[:])
                nc.vector.reciprocal(tmp2[:], tmp2[:])
                nc.vector.tensor_tensor(out=scl[:],
                                        in0=bng_sb[:, layer:layer + 1],
                                        in1=tmp2[:], op=ALU.mult)
                nc.vector.tensor_tensor(out=tmp1[:], in0=tmp1[:], in1=scl[:],
                                        op=ALU.mult)
                nc.vector.tensor_tensor(out=bia[:],
                                        in0=bnb_sb[:, layer:layer + 1],
                                        in1=tmp1[:], op=ALU.subtract)
                def wb_tile(t):
                    pt2 = psT.tile([P, P], F16, tag='trf16')
                    nc.tensor.transpose(pt2[:], nxt[:, t * P:(t + 1) * P],
                                        ident_sb[:])
                    et = epool.tile([P, P], F16, tag='ev')
                    nc.scalar.copy(et[:], pt2[:])
                    nc.sync.dma_start(hown_d[t * P:(t + 1) * P, :], et[:])

                t_done = 0
                for g in range(NGRP):
                    w = grp_w[g]
                    pt = dense_prebn(layer, g, cur)
                    sl = slice(g * 512, g * 512 + w)
                    nc.scalar.activation(nxt[:, sl], pt[:, :w], ACTF.Relu,
                                         bias=bia[:], scale=scl[:])
                    if residual:
                        nc.vector.tensor_tensor(out=nxt[:, sl],
                                                in0=nxt[:, sl],
                                                in1=cur[:, sl], op=ALU.add)
                    if writeback:
                        lim = ((g + 1) * 512) // P
                        if g == NGRP - 1:
                            if pl.OWNP > pl.OWN:
                                nc.vector.memset(
                                    nxt[:, pl.OWN:pl.OWNP], 0.0)
                            lim = pl.T
                        while t_done < min(lim, pl.T):
                            wb_tile(t_done)
                            t_done += 1
                if not writeback and pl.OWNP > pl.OWN:
                    nc.vector.memset(nxt[:, pl.OWN:pl.OWNP], 0.0)
                if writeback:
                    nc.gpsimd.collective_compute(
                        'AllGather', ALU.bypass, rg, ins=[hown_d[:]],
                        outs=[hfull_d[:]])

            # layer 0: SAGE(x)
            aggregate(xpad_d, gin=False, cur_h=None,
                      group_cb=stats_pass1(0, None))
            bn_stats_and_apply(0, cur=None, nxt=hA, residual=False)
            # layer 1: GIN
            _p1_gin = stats_pass1(1, hA)

            def gin_cb(g):
                w = grp_w[g]
                pt = psD.tile([P, 512], F32, tag='dense')
                sl = slice(g * 512, g * 512 + w)
                nc.tensor.matmul(pt[:, :w], lhsT=w_sb['w1'][:],
                                 rhs=aggT[:, sl], start=True, stop=True,
                                 skip_group_check=True)
                nc.scalar.activation(xq[:, sl], pt[:, :w], ACTF.Relu,
                                     bias=b1_sb[:], scale=1.0)
                _p1_gin(g)

            aggregate(hfull_d, gin=True, cur_h=hA, group_cb=gin_cb)
            bn_stats_and_apply(1, cur=hA, nxt=hB, residual=True)
            # layer 2: SAGE
            aggregate(hfull_d, gin=False, cur_h=None,
                      group_cb=stats_pass1(2, hB))
            bn_stats_and_apply(2, cur=hB, nxt=hA, residual=True)
            # final SAGE + log_softmax
            def fin_cb(g):
                for t in range(g * 4, min(g * 4 + 4, pl.T)):
                    pt = psD.tile([P, DOUT], F32, tag='dense')
                    nc.tensor.matmul(pt[:, :DOUT],
                                     lhsT=aggT[:, t * P:(t + 1) * P],
                                     rhs=fwl_sb[:], start=True, stop=False,
                                     skip_group_check=True)
                    nc.tensor.matmul(pt[:, :DOUT],
                                     lhsT=hA[:, t * P:(t + 1) * P],
                                     rhs=fwr_sb[:], start=False, stop=True,
                                     skip_group_check=True)
                    nc.vector.tensor_tensor(
                        out=logit[:, t * DOUT:(t + 1) * DOUT],
                        in0=pt[:, :DOUT], in1=finb_sb[:], op=ALU.add)

            aggregate(hfull_d, gin=False, cur_h=None, group_cb=fin_cb)
            lv = logit[:].rearrange('p (t c) -> p t c', t=pl.T)
            nc.vector.tensor_reduce(out=mx[:], in_=lv, axis=AX.X, op=ALU.max)
            nc.vector.tensor_tensor(
                out=lv, in0=lv, in1=_bcast_last(mx[:], DOUT),
                op=ALU.subtract)
            for t in range(pl.T):
                nc.scalar.activation(
                    escr[:], logit[:, t * DOUT:(t + 1) * DOUT], ACTF.Exp,
                    accum_out=lse[:, t:t + 1])
            nc.scalar.activation(lse[:], lse[:], ACTF.Ln)
            nc.vector.tensor_tensor(
                out=lv, in0=lv, in1=_bcast_last(lse[:], DOUT),
                op=ALU.subtract)
            for t in range(pl.T):
                nc.sync.dma_start(
                    out_d[t * P:(t + 1) * P, :DOUT],
                    logit[:, t * DOUT:(t + 1) * DOUT])

    lower_extended_insts(nc)
    _split_sync_waits(nc)
    return nc


def _make_weight_arrays(inp):
    f16 = np.float16
    return {
        'w0l': np.asarray(inp['sage0_wl'], np.float32).astype(f16),
        'w0r': np.asarray(inp['sage0_wr'], np.float32).astype(f16),
        'w1': np.asarray(inp['gin_w1'], np.float32).astype(f16),
        'w2': np.asarray(inp['gin_w2'], np.float32).astype(f16),
        'w2l': np.asarray(inp['sage2_wl'], np.float32).astype(f16),
        'w2r': np.asarray(inp['sage2_wr'], np.float32).astype(f16),
        'fwl': np.asarray(inp['fin_wl'], np.float32).astype(f16),
        'fwr': np.asarray(inp['fin_wr'], np.float32).astype(f16),
        'b1T': np.asarray(inp['gin_b1'], np.float32).reshape(P, 1),
        'finb': np.tile(np.asarray(inp['fin_b'], np.float32)[None, :],
                        (P, 1)),
        'bngT': np.ascontiguousarray(
            np.asarray(inp['bn_gamma'], np.float32).T),
        'bnbT': np.ascontiguousarray(
            np.asarray(inp['bn_beta'], np.float32).T),
        'iota': np.tile(np.arange(P, dtype=np.float32)[None, :],
                        (P, 1)).astype(f16),
        'ident': np.eye(P, dtype=np.float32).astype(f16),
    }


_CACHE = {}


def _build_and_run(inputs, trace=False):
    _apply_tile_drain_patch()
    _install_ntff_hook()
    _maybe_reset_device()
    from concourse.bass_utils import run_bass_kernel_spmd

    x = np.asarray(inputs['x'], np.float32)
    ei = np.asarray(inputs['edge_index'])
    plan = build_plan(x, ei)
    w = _make_weight_arrays(inputs)
    nc = build_bass(plan)

    in_maps = []
    for c in range(NCORES):
        m = {
            'xpad': np.asarray(plan.xpad),
            'xT': plan.xT[c],
            'idxw': plan.idx_w[c],
            'ds': plan.ds[c],
            'invdeg': plan.invdeg[c],
        }
        m.update({k: w[k] for k in
                  ['iota', 'ident', 'w0l', 'w0r', 'w1', 'w2', 'w2l', 'w2r',
                   'fwl', 'fwr', 'b1T', 'finb', 'bngT', 'bnbT']})
        in_maps.append(m)
    res = run_bass_kernel_spmd(nc, in_maps, core_ids=list(range(NCORES)),
                               trace=trace)
    outs = [res.results[c]['out'][:plan.OWN, :DOUT] for c in range(NCORES)]
    return np.concatenate(outs, axis=0).astype(np.float32), res


def kernel(**inputs):
    out, _ = _build_and_run(inputs, trace=False)
    return out


def kernel_traced(**inputs):
    return _build_and_run(inputs, trace=True)

